# revision 15
# baseline (speedup 1.0000x reference)
"""MGDPR (gnn_message_passing) Trainium2 kernel, 8 NeuronCores.

Sharding: nodes row-sharded 4-way within each batch element; cores 0-3 own
batch 0, cores 4-7 own batch 1 (375 nodes each, padded to 384). The source
(m) axis uses a padded block order: rank k's nodes sit at m = k*384 + j, so
the AllGather output maps 1:1 onto SBUF m-tiles. adj is host-cast to fp8e4
and pre-laid in the exact SBUF layout ([mi][mt, r, j]) so the load is a few
large fully-contiguous DMAs. All per-node tensors are channel-major on chip
([C, nodes]); h is kept bf16. The embedding is fused into layer-0 diffusion
mt-major so compute rides the adjacency DMA stream. Between layers, h is
gathered c-major over the 4 cores of each batch (bf16, one AllGather), and
m-tiles are rebuilt with 12 DMA-transposes (xbar). GroupNorm's affine
(gn_g/gn_b) is folded into w2/bias on the host; h_prime's contribution
(zeros init + affine) folds into a per-layer bias.
"""

import numpy as np

try:
    import concourse.bass as bass
except ImportError:
    import sys

    sys.path.insert(0, "/opt/trn_rl_repo")
    import concourse.bass as bass

import ml_dtypes
import concourse.mybir as mybir
import concourse.tile as tile
from concourse import bacc
from concourse.bass_utils import run_bass_kernel_spmd

B, N, T, DIN, C, R, K, L, H, OUT = 2, 1500, 20, 32, 128, 5, 5, 3, 4, 2
HD = C // H
EPS = 1e-5
NCORES = 8
NS = 375            # real nodes per core
NSP = 384           # padded nodes per core (3 * 128)
NT = 12             # m tiles: 4 ranks * 3 tiles
MPAD = NT * 128     # 1536 = 4 * NSP
RG = [[0, 1, 2, 3], [4, 5, 6, 7]]
F32 = mybir.dt.float32
F32R = mybir.dt.float32r
BF16 = mybir.dt.bfloat16
FP8 = mybir.dt.float8e4
NCOL = 6 * L + 3    # bias columns
PACKW = 3 * 128 + L * 4 * 128 + L * R * 128 + L * 128 + 128 + OUT  # packed consts

_NC_CACHE = {}


def _build_nc():
    if "nc" in _NC_CACHE:
        return _NC_CACHE["nc"]
    nc = bacc.Bacc(None, target_bir_lowering=False, debug=False, num_devices=NCORES)

    adjt = nc.dram_tensor("adjt", [128, NT * R * NSP], FP8, kind="ExternalInput")
    xt = nc.dram_tensor("xt", [DIN + 1, MPAD], BF16, kind="ExternalInput")
    embt_d = nc.dram_tensor("embt", [DIN + 1, C], BF16, kind="ExternalInput")
    pack_d = nc.dram_tensor("pack", [C, PACKW], BF16, kind="ExternalInput")
    cols_d = nc.dram_tensor("cols", [C, NCOL], F32, kind="ExternalInput")

    outt = nc.dram_tensor("outt", [OUT, NSP], F32R, kind="ExternalOutput")

    g_in = [nc.dram_tensor(f"g_in_{l}", [C, NSP], BF16) for l in range(2)]
    g_out = [nc.dram_tensor(f"g_out_{l}", [4, C, NSP], BF16) for l in range(2)]

    with tile.TileContext(nc) as tc:
        with (
            tc.tile_pool(name="persist", bufs=1) as pers,
            tc.tile_pool(name="work", bufs=2) as work,
            tc.tile_pool(name="zwork", bufs=2) as zwork,
            tc.tile_pool(name="small", bufs=2) as small,
            tc.tile_pool(name="pz", bufs=2, space="PSUM") as pz,
            tc.tile_pool(name="pp", bufs=4, space="PSUM") as pp,
            tc.tile_pool(name="pm", bufs=1, space="PSUM") as pm,
        ):
            # ---------- resident tensors ----------
            adjsb = pers.tile([128, NT, R, NSP], FP8, tag="adjsb")
            hnf8 = pers.tile([128, NT * 128], FP8, tag="hnf8")
            xtsb = pers.tile([DIN + 1, MPAD], BF16, tag="xtsb")
            embtsb = pers.tile([DIN + 1, C], BF16, tag="embtsb")
            packsb = pers.tile([C, PACKW], BF16, tag="packsb")
            colsb = pers.tile([C, NCOL], F32, tag="colsb")
            o = 0
            indbsb = packsb[:, o : o + 128]; o += 128
            indssb = packsb[:, o : o + 128]; o += 128
            identsb = packsb[:, o : o + 128]; o += 128
            qkvosb = packsb[:, o : o + L * 4 * 128]; o += L * 4 * 128
            wpsb = packsb[:, o : o + L * R * 128]; o += L * R * 128
            w2atsb = packsb[:, o : o + L * 128]; o += L * 128
            ow1tsb = packsb[:, o : o + 128]; o += 128
            ow2tsb = packsb[:, o : o + OUT]; o += OUT

            # embt + cols first, then xt/adj, then the packed consts
            nc.scalar.dma_start(embtsb[:], embt_d[:, :])
            nc.scalar.dma_start(colsb[:], cols_d[:, :])
            nc.sync.dma_start(xtsb[:], xt[:, :])
            adjflat = adjsb.rearrange("p a b c -> p (a b c)")
            CH = 4 * R * NSP
            engs = [nc.sync, nc.scalar, nc.gpsimd]
            for ch in range(NT // 4):
                engs[ch].dma_start(
                    adjflat[:, ch * CH : (ch + 1) * CH],
                    adjt[:, ch * CH : (ch + 1) * CH],
                )
            nc.scalar.dma_start(packsb[:], pack_d[:, :])

            def col(i):
                return colsb[:, i : i + 1]

            wp4 = wpsb.rearrange("p (l r co) -> p l r co", l=L, r=R)
            qk4 = qkvosb.rearrange("p (l i co) -> p l i co", l=L, i=4)
            w2a3 = w2atsb.rearrange("p (l co) -> p l co", l=L)
            identsb = identsb.opt()

            def embed(mt):
                ep = pp.tile([128, 128], F32, tag="pp")
                nc.tensor.matmul(
                    ep[:], xtsb[:, mt * 128 : (mt + 1) * 128], embtsb[:],
                    start=True, stop=True, skip_group_check=True,
                )
                with nc.allow_low_precision(reason="fp8 h for diffusion"):
                    if mt % 2 == 0:
                        nc.vector.tensor_copy(
                            hnf8[:, mt * 128 : (mt + 1) * 128], ep[:]
                        )
                    else:
                        nc.scalar.copy(hnf8[:, mt * 128 : (mt + 1) * 128], ep[:])

            for l in range(L):
                # ---- diffusion, r-outer, DoubleRow pairs ----
                if l == 0:
                    embed(0)
                    embed(1)
                if l == 0:
                    for mt in range(2, NT):
                        embed(mt)
                h3 = hnf8.rearrange("p (mt c) -> p mt c", mt=NT)
                mps = pm.tile([128, NSP], F32, tag="m")
                for r in range(R):
                    zp = pz.tile([128, NSP], F32, tag="z", name=f"zp_{l}_{r}")
                    for m2 in range(NT // 2):
                        nc.tensor.matmul(
                            zp[:],
                            h3[:, 2 * m2 : 2 * m2 + 2, :],
                            adjsb[:, 2 * m2 : 2 * m2 + 2, r, :],
                            start=(m2 == 0), stop=(m2 == NT // 2 - 1),
                            perf_mode=mybir.MatmulPerfMode.DoubleRow,
                            skip_group_check=True,
                        )
                    zsb = zwork.tile([128, NSP], BF16, tag="zsb")
                    if r % 2 == 0:
                        nc.scalar.copy(zsb[:], zp[:])
                    else:
                        nc.vector.tensor_copy(zsb[:], zp[:])
                    nc.tensor.matmul(
                        mps[:], wp4[:, l, r, :], zsb[:],
                        start=(r == 0), stop=(r == R - 1),
                        skip_group_check=True,
                    )
                hdT = work.tile([128, NSP], BF16, tag="hdT")
                nc.scalar.activation(
                    hdT[:], mps[:], mybir.ActivationFunctionType.Relu,
                    bias=col(6 * l + 5), scale=1.0,
                )

                # ---- retention (S=1), channel-major, block matmuls,
                # two column halves stage-interleaved to fill PE gaps ----
                HW_ = NSP // 2
                sls = [slice(0, HW_), slice(HW_, NSP)]
                qsb = work.tile([128, NSP], BF16, tag="qsb")
                ksb = work.tile([128, NSP], BF16, tag="ksb")
                vsb = work.tile([128, NSP], BF16, tag="vsb")
                qk = work.tile([128, NSP], BF16, tag="qk")
                osb = work.tile([128, NSP], BF16, tag="osb")
                o2sb = work.tile([128, NSP], BF16, tag="o2sb")
                ctr = work.tile([128, NSP], BF16, tag="ctr")
                d2 = work.tile([128, NSP], BF16, tag="d2")
                stdf = work.tile([128, NSP], BF16, tag="stdf")
                rstdf = work.tile([128, NSP], BF16, tag="rstdf")
                hrT = work.tile([128, NSP], BF16, tag="hrT")
                hnT = work.tile([128, NSP], BF16, tag="hnT")

                qps, kps, vps = [], [], []
                for hf in range(2):
                    s = sls[hf]
                    qp = pp.tile([128, HW_], F32, tag="pp", name=f"qp{l}{hf}")
                    nc.tensor.matmul(qp[:], qk4[:, l, 0, :], hdT[:, s], start=True, stop=True)
                    kp = pp.tile([128, HW_], F32, tag="pp", name=f"kp{l}{hf}")
                    nc.tensor.matmul(kp[:], qk4[:, l, 1, :], hdT[:, s], start=True, stop=True)
                    qps.append(qp); kps.append(kp)
                for hf in range(2):
                    s = sls[hf]
                    nc.scalar.activation(
                        qsb[:, s], qps[hf][:], mybir.ActivationFunctionType.Identity,
                        bias=col(6 * l + 0),
                    )
                    nc.vector.tensor_scalar_add(ksb[:, s], kps[hf][:], col(6 * l + 1))
                for hf in range(2):
                    s = sls[hf]
                    nc.vector.tensor_mul(qk[:, s], qsb[:, s], ksb[:, s])
                    vp = pp.tile([128, HW_], F32, tag="pp", name=f"vp{l}{hf}")
                    nc.tensor.matmul(vp[:], qk4[:, l, 2, :], hdT[:, s], start=True, stop=True)
                    vps.append(vp)
                sbps = []
                for hf in range(2):
                    s = sls[hf]
                    sb = pp.tile([128, HW_], F32, tag="pp", name=f"sb{l}{hf}")
                    nc.tensor.matmul(sb[:], indbsb[:], qk[:, s], start=True, stop=True)
                    sbps.append(sb)
                    nc.scalar.activation(
                        vsb[:, s], vps[hf][:], mybir.ActivationFunctionType.Identity,
                        bias=col(6 * l + 2),
                    )
                for hf in range(2):
                    s = sls[hf]
                    nc.vector.tensor_mul(osb[:, s], vsb[:, s], sbps[hf][:])
                o2ps = []
                for hf in range(2):
                    s = sls[hf]
                    o2p = pp.tile([128, HW_], F32, tag="pp", name=f"o2p{l}{hf}")
                    nc.tensor.matmul(o2p[:], qk4[:, l, 3, :], osb[:, s], start=True, stop=True)
                    o2ps.append(o2p)
                for hf in range(2):
                    s = sls[hf]
                    if hf == 0:
                        nc.vector.tensor_copy(o2sb[:, s], o2ps[hf][:])
                    else:
                        nc.scalar.copy(o2sb[:, s], o2ps[hf][:])
                mups = []
                for hf in range(2):
                    s = sls[hf]
                    mu = pp.tile([128, HW_], F32, tag="pp", name=f"mu{l}{hf}")
                    nc.tensor.matmul(mu[:], indssb[:], o2sb[:, s], start=True, stop=True)
                    mups.append(mu)
                for hf in range(2):
                    s = sls[hf]
                    nc.vector.scalar_tensor_tensor(
                        ctr[:, s], o2sb[:, s], col(6 * l + 3), mups[hf][:],
                        mybir.AluOpType.add, mybir.AluOpType.subtract,
                    )
                    nc.vector.tensor_mul(d2[:, s], ctr[:, s], ctr[:, s])
                vrps = []
                for hf in range(2):
                    s = sls[hf]
                    vr = pp.tile([128, HW_], F32, tag="pp", name=f"vr{l}{hf}")
                    nc.tensor.matmul(vr[:], indssb[:], d2[:, s], start=True, stop=True)
                    vrps.append(vr)
                for hf in range(2):
                    s = sls[hf]
                    nc.scalar.activation(
                        stdf[:, s], vrps[hf][:], mybir.ActivationFunctionType.Sqrt,
                        bias=col(6 * L + 2),
                    )
                    with nc.allow_low_precision(reason="groupnorm rstd in bf16"):
                        nc.vector.reciprocal(rstdf[:, s], stdf[:, s])
                for hf in range(2):
                    s = sls[hf]
                    nc.vector.tensor_mul(hrT[:, s], ctr[:, s], rstdf[:, s])

                # h update (gn affine + h_prime folded on host)
                for hf in range(2):
                    s = sls[hf]
                    h2p = pp.tile([128, HW_], F32, tag="pp", name=f"h2p{l}{hf}")
                    nc.tensor.matmul(h2p[:], w2a3[:, l, :], hrT[:, s], start=True, stop=True)
                    nc.scalar.activation(
                        hnT[:, s], h2p[:], mybir.ActivationFunctionType.Relu,
                        bias=col(6 * l + 4),
                    )

                if l < 2:
                    # c-major gather; input DMA'd per half as it completes
                    nc.sync.dma_start(g_in[l][:, : NSP // 2], hnT[:, : NSP // 2])
                    nc.sync.dma_start(g_in[l][:, NSP // 2 :], hnT[:, NSP // 2 :])
                    nc.gpsimd.collective_compute(
                        "AllGather", mybir.AluOpType.bypass,
                        replica_groups=RG,
                        ins=[g_in[l][:, :].opt()],
                        outs=[g_out[l][:, :, :].opt()],
                    )
                    gsb = work.tile([128, 4 * NSP], BF16, tag="gsb")
                    for kk in range(4):
                        eng = nc.sync if kk % 2 == 0 else nc.scalar
                        eng.dma_start(
                            gsb[:, kk * NSP : (kk + 1) * NSP], g_out[l][kk, :, :]
                        )
                    with nc.allow_low_precision(reason="fp8 h for diffusion"):
                        for t in range(NT):
                            trp = pp.tile([128, 128], BF16, tag="pp", name=f"trp_{l}_{t}")
                            nc.tensor.transpose(
                                trp[:], gsb[:, t * 128 : (t + 1) * 128], identsb[:]
                            )
                            if t % 2 == 0:
                                nc.vector.tensor_copy(
                                    hnf8[:, t * 128 : (t + 1) * 128], trp[:]
                                )
                            else:
                                nc.scalar.copy(
                                    hnf8[:, t * 128 : (t + 1) * 128], trp[:]
                                )
                else:
                    hmsb = work.tile([128, NSP], BF16, tag="hmsb")
                    oosb = small.tile([OUT, NSP], F32R, tag="oosb")
                    for hf in range(2):
                        s = sls[hf]
                        hmp = pp.tile([128, HW_], F32, tag="pp", name=f"hmp{hf}")
                        nc.tensor.matmul(
                            hmp[:], ow1tsb[:], hnT[:, s], start=True, stop=True
                        )
                        nc.scalar.activation(
                            hmsb[:, s], hmp[:], mybir.ActivationFunctionType.Relu,
                            bias=col(6 * L),
                        )
                        oop = pp.tile([OUT, HW_], F32, tag="pp", name=f"oop{hf}")
                        nc.tensor.matmul(
                            oop[:], ow2tsb[:], hmsb[:, s], start=True, stop=True
                        )
                        nc.scalar.activation(
                            oosb[:, s], oop[:], mybir.ActivationFunctionType.Identity,
                            bias=colsb[0:OUT, 6 * L + 1 : 6 * L + 2],
                        )
                    nc.sync.dma_start(outt[:, :], oosb[:])

    nc.finalize()
    _NC_CACHE["nc"] = nc
    return nc


def _prep(inputs):
    f32 = np.float32

    def g(name):
        return np.asarray(inputs[name], f32)

    x, adj = g("x"), g("adj_list")
    alpha, transition = g("alpha"), g("transition")
    conv_w, conv_b = g("conv_w"), g("conv_b")
    w1, b1, eb1 = g("w1"), g("b1"), g("eb1")
    w2, b2, eb2 = g("w2"), g("b2"), g("eb2")
    gn_g, gn_b = g("gn_g"), g("gn_b")

    a = alpha - alpha.max(-1, keepdims=True)
    e = np.exp(a)
    srow = (e / e.sum(-1, keepdims=True)).sum(-1)          # [L,R]
    Wm = transition.mean(axis=2)                            # [L,R,C,C]
    Wp = (conv_w * srow)[:, :, None, None] * np.swapaxes(Wm, -1, -2)

    # h_prime path and groupnorm affine folded into the layer bias
    hp = np.zeros((C,), f32)
    b2eff = np.zeros((L, C), f32)
    for l in range(L):
        b2eff[l] = b2[l] + eb2[l] + w2[l][:, C:] @ hp + w2[l][:, :C] @ gn_b[l]
        hp = np.maximum(hp @ w1[l].T + b1[l] + eb1[l], 0.0).astype(f32)

    qkvo = np.stack(
        [np.swapaxes(g(w), -1, -2) for w in ("qw", "kw", "vw", "ow")], axis=1
    )  # [L,4,C,C] lhsT layout

    # w2a with gn_g folded: lhsT[c, o] = w2[l, o, c] * gn_g[l, c]
    w2at = np.swapaxes(w2[:, :, :C], -1, -2) * gn_g[:, :, None]  # [L,C,C]

    hid = np.arange(C) // HD
    same = (hid[:, None] == hid[None, :]).astype(f32)          # [C,C] same-head

    # centered ob: the part of ob surviving groupnorm mean subtraction
    ob = g("ob")
    obc = ob - ob @ (same / HD).T                               # [L,C]

    cols = np.zeros((C, NCOL), f32)
    for l in range(L):
        cols[:, 6 * l + 0] = g("qb")[l]
        cols[:, 6 * l + 1] = g("kb")[l]
        cols[:, 6 * l + 2] = g("vb")[l]
        cols[:, 6 * l + 3] = obc[l]
        cols[:, 6 * l + 4] = b2eff[l]
        cols[:, 6 * l + 5] = conv_b[l]
    cols[:, 6 * L] = g("out_b1")
    cols[:OUT, 6 * L + 1] = g("out_b2")
    cols[:, 6 * L + 2] = EPS

    bf = ml_dtypes.bfloat16
    pack = np.concatenate(
        [
            same,
            same / HD,
            np.eye(C, dtype=f32),
            np.ascontiguousarray(qkvo.transpose(2, 0, 1, 3)).reshape(C, L * 4 * C),
            np.ascontiguousarray(Wp.transpose(2, 0, 1, 3)).reshape(C, L * R * C),
            np.ascontiguousarray(w2at.transpose(1, 0, 2)).reshape(C, L * C),
            np.ascontiguousarray(g("out_w1").T),
            np.ascontiguousarray(g("out_w2").T),
        ],
        axis=1,
    )
    assert pack.shape == (C, PACKW)
    consts = {
        "embt": np.concatenate([g("emb_w").T, g("emb_b")[None, :]], axis=0).astype(bf),
        "pack": pack.astype(bf),
        "cols": cols,
    }

    xlast = x[:, :, -1, :]                                   # [B,N,DIN]
    fp8 = ml_dtypes.float8_e4m3
    in_maps = []
    for core in range(NCORES):
        b, s = core // 4, core % 4
        n0 = s * NS
        # adj block layout: A[m_pad, r, j] with m_pad = rank*384 + jj
        Ab = np.zeros((R, NSP, 4, NSP), f32)                 # [r, j, rank, jj]
        for s2 in range(4):
            Ab[:, :NS, s2, :NS] = adj[b][:, n0 : n0 + NS, s2 * NS : (s2 + 1) * NS]
        a3 = (
            Ab.transpose(2, 3, 0, 1)                          # [rank, jj, r, j]
            .reshape(MPAD, R, NSP)
            .reshape(NT, 128, R, NSP)
            .transpose(1, 0, 2, 3)                            # [mi, mt, r, j]
        )
        xtc = np.zeros((DIN + 1, MPAD), f32)
        xtc[DIN, :] = 1.0
        xv = xlast[b].T                                       # [DIN, N]
        for s2 in range(4):
            xtc[:DIN, s2 * NSP : s2 * NSP + NS] = xv[:, s2 * NS : (s2 + 1) * NS]
        in_maps.append(
            dict(
                consts,
                adjt=np.ascontiguousarray(a3).reshape(128, NT * R * NSP).astype(fp8),
                xt=xtc.astype(bf),
            )
        )
    return in_maps


def kernel(**inputs):
    nc = _build_nc()
    in_maps = _prep(inputs)
    res = run_bass_kernel_spmd(nc, in_maps, core_ids=list(range(NCORES)))
    out = np.zeros((B, N, OUT), np.float32)
    for core in range(NCORES):
        b, s = core // 4, core % 4
        out[b, s * NS : (s + 1) * NS, :] = np.asarray(
            res.results[core]["outt"], np.float32
        )[:, :NS].T
    return out
